# revision 17
# baseline (speedup 1.0000x reference)
"""Multi-head self-attention (B=4, N=4096, C=256, nh=8) on 8 trn2 NeuronCores.

Sharding: core c handles batch b=c//2 and query-half qh=c%2 (2048 q tokens,
all 8 heads, full 4096-token K/V context for the batch). Outputs are disjoint
slices of the final [4, 4096, 256] tensor, so gathering is host-side
concatenation. Single SPMD Bass program; per-core behavior comes purely from
per-core input data (no runtime core-id branching).

Per-core pipeline:
  1. Load xs^T (token-transposed host-side), weights (host-pretransposed),
     biases (host-prebroadcast).
  2. Projections on PE: Q^T, K^T as [cout, tokens]; V token-major with an
     interleaved ones-column per head (softmax-denominator trick); vres =
     V projection of the q-half (residual path).
  3. Attention per (head-block pb, q-chunk qc, k-block kb):
       S^T[k,q] = K_h @ Q_h^T on PE, 4 heads packed via tile_position row
                  tiling (contraction K=32 uses 32-row strips of the array)
       P^T = exp(S^T * 1/sqrt(dh)) on ScalarE, PSUM -> SBUF, one op per
             k-block spanning the 4 heads' PSUM banks (free dim 2048)
       O^T[d,q] += [V_h | 1]^T @ P_h^T on PE; row 32 accumulates the softmax
                  denominator for free
     Normalize: DVE reciprocal(denom row) -> GPSIMD partition_broadcast ->
     DVE multiply into per-head attnout tiles.
  4. Output projection (8 per-head K=32 matmuls accumulating in PSUM),
     + residual(vres) + bias, LayerNorm over C on DVE with one batched Sqrt
     on ScalarE (single activation-table switch).
"""

import os
from contextlib import ExitStack

import numpy as np

import concourse.bass as bass
import concourse.tile as tile
from concourse import mybir
from concourse.bass_utils import run_bass_kernel_spmd

F32 = mybir.dt.float32
F32R = mybir.dt.float32r
AF = mybir.ActivationFunctionType
ALU = mybir.AluOpType

B, HH, WW, C = 4, 64, 64, 256
N = HH * WW          # 4096 tokens per batch
NH, DH = 8, 32       # heads, head dim
NCORES = 8
NQ = N // 2          # 2048 q tokens per core
PB = 2               # partition blocks of C (2 x 128)
P = 128
SCALE = 1.0 / float(np.sqrt(DH))
LN_EPS = 1e-6

QC = 512             # q chunk (matmul free dim)
NQC = NQ // QC       # 4
NKB = N // P         # 32 k blocks
NTB = N // P         # 32 token blocks (full batch)
NQB = NQ // P        # 16 token blocks (q half)
VST = NH * 33        # 264: V block layout [128 tok, 8 heads x (32 | ones)]

# Fast matmul mode: stream fp32 operands as float32r (1 cyc/row vs 4).
FAST_MM = os.environ.get("BASS_FAST_MM", "1") == "1"


DT = F32R if FAST_MM else F32   # dtype of matmul-feeding tensors


def build_program():
    nc = bass.Bass()

    d_xsT = nc.declare_dram_parameter("xsT", [C, N], DT, isOutput=False)
    d_xsTq = nc.declare_dram_parameter("xsTq", [C, NQ], DT, isOutput=False)
    d_wqT = nc.declare_dram_parameter("wqT", [C, C], DT, isOutput=False)
    d_wkT = nc.declare_dram_parameter("wkT", [C, C], DT, isOutput=False)
    d_wvT = nc.declare_dram_parameter("wvT", [C, C], DT, isOutput=False)
    d_wvTp = nc.declare_dram_parameter("wvT_pad", [C, VST], DT, isOutput=False)
    d_bvp = nc.declare_dram_parameter("bv_pad_bc", [P, VST], F32, isOutput=False)
    d_woT = nc.declare_dram_parameter("woT_ph", [DH, NH * C], DT, isOutput=False)
    d_bq = nc.declare_dram_parameter("bq2", [P, PB], F32, isOutput=False)
    d_bk = nc.declare_dram_parameter("bk2", [P, PB], F32, isOutput=False)
    d_bv = nc.declare_dram_parameter("bv_bc", [P, C], F32, isOutput=False)
    d_bo = nc.declare_dram_parameter("bo_bc", [P, C], F32, isOutput=False)
    d_gamma = nc.declare_dram_parameter("gamma_bc", [P, C], F32, isOutput=False)
    d_beta = nc.declare_dram_parameter("beta_bc", [P, C], F32, isOutput=False)
    d_out = nc.declare_dram_parameter("out", [NQ, C], F32, isOutput=True)

    with tile.TileContext(nc) as tc, ExitStack() as ctx:
        singles = ctx.enter_context(tc.tile_pool(name="singles", bufs=1))

        wqT_sb = singles.tile([P, PB * C], DT, name="wqT_sb")
        wkT_sb = singles.tile([P, PB * C], DT, name="wkT_sb")
        wvT_sb = singles.tile([P, PB * C], DT, name="wvT_sb")
        wvTp_sb = singles.tile([P, PB * VST], DT, name="wvTp_sb")
        bvp_sb = singles.tile([P, VST], F32, name="bvp_sb")
        woT_sb = singles.tile([DH, NH * C], DT, name="woT_sb")
        bq_sb = singles.tile([P, PB], F32, name="bq_sb")
        bk_sb = singles.tile([P, PB], F32, name="bk_sb")
        bv_sb = singles.tile([P, C], F32, name="bv_sb")
        bo_sb = singles.tile([P, C], F32, name="bo_sb")
        gamma_sb = singles.tile([P, C], F32, name="gamma_sb")
        beta_sb = singles.tile([P, C], F32, name="beta_sb")

        qT_sb = singles.tile([P, PB * NQ], DT, name="qT_sb")
        kT_sb = singles.tile([P, PB * N], DT, name="kT_sb")
        v_sb = singles.tile([P, NTB * VST], DT, name="v_sb")
        vres_sb = singles.tile([P, NQB * C], F32, name="vres_sb")
        stats_sb = singles.tile([P, NQB], F32, name="stats_sb")
        invstd_sb = singles.tile([P, NQB], F32, name="invstd_sb")
        eps_sb = singles.tile([P, 1], F32, name="eps_sb")
        nc.vector.memset(eps_sb[:], LN_EPS)

        for pb in range(PB):
            nc.sync.dma_start(wqT_sb[:, pb * C:(pb + 1) * C], d_wqT[pb * P:(pb + 1) * P, :])
            nc.sync.dma_start(wkT_sb[:, pb * C:(pb + 1) * C], d_wkT[pb * P:(pb + 1) * P, :])
            nc.sync.dma_start(wvT_sb[:, pb * C:(pb + 1) * C], d_wvT[pb * P:(pb + 1) * P, :])
            nc.sync.dma_start(wvTp_sb[:, pb * VST:(pb + 1) * VST], d_wvTp[pb * P:(pb + 1) * P, :])
        nc.sync.dma_start(woT_sb[:], d_woT[:])
        nc.sync.dma_start(bq_sb[:], d_bq[:])
        nc.sync.dma_start(bk_sb[:], d_bk[:])
        nc.sync.dma_start(bv_sb[:], d_bv[:])
        nc.sync.dma_start(bvp_sb[:], d_bvp[:])
        nc.sync.dma_start(bo_sb[:], d_bo[:])
        nc.sync.dma_start(gamma_sb[:], d_gamma[:])
        nc.sync.dma_start(beta_sb[:], d_beta[:])

        # ones-columns come from the padded V projection: wvT_pad has a zero
        # column at each head's 33rd slot and bv_pad_bc carries 1.0 there.

        # ---- phase 1: projections (xs pool released afterwards) ----
        with tc.tile_pool(name="xs_pool", bufs=1) as xs_pool, \
             tc.tile_pool(name="proj_psum", bufs=2, space="PSUM") as proj_psum:
            xsT_sb = xs_pool.tile([P, PB * N], DT, name="xsT_sb")
            xsTq_sb = xs_pool.tile([P, PB * NQ], DT, name="xsTq_sb")
            for pb in range(PB):
                nc.sync.dma_start(xsT_sb[:, pb * N:(pb + 1) * N],
                                  d_xsT[pb * P:(pb + 1) * P, :])
                nc.sync.dma_start(xsTq_sb[:, pb * NQ:(pb + 1) * NQ],
                                  d_xsTq[pb * P:(pb + 1) * P, :])

            # K^T and Q^T: [cout(part), tokens]
            for (wsb, bsb, dst, ntok, src) in (
                (wkT_sb, bk_sb, kT_sb, N, xsT_sb),
                (wqT_sb, bq_sb, qT_sb, NQ, xsTq_sb),
            ):
                for pbo in range(PB):
                    for t0 in range(0, ntok, QC):
                        pt = proj_psum.tile([P, QC], F32, name="pt", tag="pt")
                        for pbi in range(PB):
                            nc.tensor.matmul(
                                pt[:],
                                (wsb[:, pbi * C + pbo * P: pbi * C + (pbo + 1) * P]),
                                (src[:, pbi * ntok + t0: pbi * ntok + t0 + QC]),
                                start=(pbi == 0), stop=(pbi == PB - 1),
                            )
                        nc.vector.tensor_scalar_add(
                            dst[:, pbo * ntok + t0: pbo * ntok + t0 + QC],
                            pt[:], bsb[:, pbo:pbo + 1],
                        )

            # V token-major over the full batch (33-stride head layout)
            for tb in range(NTB):
                vt = proj_psum.tile([P, VST], F32, name="vt", tag="vt")
                for pbi in range(PB):
                    nc.tensor.matmul(
                        vt[:],
                        (xsT_sb[:, pbi * N + tb * P: pbi * N + (tb + 1) * P]),
                        (wvTp_sb[:, pbi * VST:(pbi + 1) * VST]),
                        start=(pbi == 0), stop=(pbi == PB - 1),
                    )
                nc.vector.scalar_tensor_tensor(
                    v_sb[:, tb * VST:(tb + 1) * VST],
                    vt[:], 0.0, bvp_sb[:], ALU.bypass, ALU.add,
                )
            # vres: V of the q-half tokens, plain token-major layout
            for tb in range(NQB):
                vr = proj_psum.tile([P, C], F32, name="vr", tag="vt")
                for pbi in range(PB):
                    nc.tensor.matmul(
                        vr[:],
                        (xsTq_sb[:, pbi * NQ + tb * P: pbi * NQ + (tb + 1) * P]),
                        (wvT_sb[:, pbi * C:(pbi + 1) * C]),
                        start=(pbi == 0), stop=(pbi == PB - 1),
                    )
                nc.vector.tensor_add(vres_sb[:, tb * C:(tb + 1) * C], vr[:], bv_sb[:])

        # ---- phase 2: attention + out-proj + residual + LN stats, per q-chunk ----
        with tc.tile_pool(name="score_psum", bufs=1, space="PSUM") as score_psum, \
             tc.tile_pool(name="pv_psum", bufs=1, space="PSUM") as pv_psum, \
             tc.tile_pool(name="p_pool", bufs=3) as p_pool, \
             tc.tile_pool(name="attn_pool", bufs=1) as attn_pool, \
             tc.tile_pool(name="norm_pool", bufs=2) as norm_pool, \
             tc.tile_pool(name="dram_pool", bufs=4, space="DRAM") as dram_pool, \
             tc.tile_pool(name="out_pool", bufs=3) as out_pool, \
             tc.tile_pool(name="xc_pool", bufs=NQB) as xc_pool:
            xcs = []
            for qc in range(NQC):
                attn_t = attn_pool.tile([DH, NH * QC], DT, name="attn_t", tag="attn")
                for pb in range(PB):
                    q0 = pb * NQ + qc * QC
                    pv = [pv_psum.tile([DH + 1, QC], F32, name=f"pv{j}", tag=f"pv{j}")
                          for j in range(4)]
                    for kb in range(NKB):
                        st = score_psum.tile([P, 4 * QC], F32, name="st", tag="st")
                        for j in range(4):
                            nc.tensor.matmul(
                                st[:, j * QC:(j + 1) * QC],
                                (kT_sb[j * DH:(j + 1) * DH,
                                         pb * N + kb * P: pb * N + (kb + 1) * P]),
                                (qT_sb[j * DH:(j + 1) * DH, q0: q0 + QC]),
                                start=True, stop=True,
                                tile_position=(j * DH, 0),
                            )
                        ptile = p_pool.tile([P, 4 * QC], DT, name="ptile", tag="ptile")
                        nc.scalar.activation(ptile[:], st[:], AF.Exp, scale=SCALE)
                        for j in range(4):
                            h = pb * 4 + j
                            nc.tensor.matmul(
                                pv[j][:],
                                (v_sb[:, kb * VST + h * 33: kb * VST + h * 33 + 33]),
                                (ptile[:, j * QC:(j + 1) * QC]),
                                start=(kb == 0), stop=(kb == NKB - 1),
                            )
                    for j in range(4):
                        h = pb * 4 + j
                        recip = norm_pool.tile([DH + 1, QC], F32, name="recip", tag="recip")
                        nc.vector.reciprocal(recip[32:33, :], pv[j][32:33, :])
                        # partition-broadcast via DRAM bounce (SBUF APs cannot
                        # have zero partition step; DRAM APs can)
                        dscr = dram_pool.tile([1, QC], F32, name="dscr", tag="dscr")
                        nc.sync.dma_start(dscr[:], recip[32:33, :])
                        bcast = norm_pool.tile([DH, QC], F32, name="bcast", tag="bcast")
                        nc.sync.dma_start(bcast[:], dscr[:].to_broadcast([DH, QC]))
                        nc.vector.tensor_mul(
                            attn_t[:, h * QC:(h + 1) * QC], pv[j][0:32, :], bcast[:],
                        )
                # out-proj + residual + LN stats for this q-chunk's 4 token blocks
                for tl in range(QC // P):
                    tb = qc * (QC // P) + tl
                    # reuse a PV psum slot (dead after normalize) for out-proj
                    ot = pv_psum.tile([P, C], F32, name="ot", tag=f"pv{tl}")
                    for h in range(NH):
                        nc.tensor.matmul(
                            ot[:],
                            (attn_t[:, h * QC + tl * P: h * QC + (tl + 1) * P]),
                            (woT_sb[:, h * C:(h + 1) * C]),
                            start=(h == 0), stop=(h == NH - 1),
                        )
                    o1 = out_pool.tile([P, C], F32, name="o1", tag="o1")
                    nc.vector.tensor_add(o1[:], ot[:], vres_sb[:, tb * C:(tb + 1) * C])
                    nc.vector.tensor_add(o1[:], o1[:], bo_sb[:])
                    sum_t = out_pool.tile([P, 1], F32, name="sum_t", tag="sum_t")
                    nc.vector.tensor_reduce(sum_t[:], o1[:], mybir.AxisListType.X, ALU.add)
                    nm = out_pool.tile([P, 1], F32, name="nm", tag="nm")
                    nc.vector.tensor_scalar_mul(nm[:], sum_t[:], -1.0 / C)
                    xc = xc_pool.tile([P, C], F32, name="xc", tag="xc")
                    nc.vector.tensor_scalar_add(xc[:], o1[:], nm[:])
                    sq = out_pool.tile([P, C], F32, name="sq", tag="sq")
                    nc.vector.scalar_tensor_tensor(
                        sq[:], xc[:], 0.0, xc[:], ALU.bypass, ALU.mult,
                        accum_out=stats_sb[:, tb:tb + 1],
                    )
                    xcs.append(xc)
            # ---- LN finalize (one batched Sqrt -> one activation-table switch) ----
            std_t = singles.tile([P, NQB], F32, name="std_t")
            nc.scalar.activation(std_t[:], stats_sb[:], AF.Sqrt,
                                 bias=eps_sb[:], scale=1.0 / C)
            nc.vector.reciprocal(invstd_sb[:], std_t[:])
            for tb in range(NQB):
                f1 = out_pool.tile([P, C], F32, name="f1", tag="f1")
                nc.vector.scalar_tensor_tensor(
                    f1[:], xcs[tb][:], invstd_sb[:, tb:tb + 1], gamma_sb[:],
                    ALU.mult, ALU.mult,
                )
                f2 = out_pool.tile([P, C], F32, name="f2", tag="f2")
                nc.vector.tensor_add(f2[:], f1[:], beta_sb[:])
                nc.sync.dma_start(d_out[tb * P:(tb + 1) * P, :], f2[:])

    # Walrus allows only 1 sync wait on self-loading (fp32/fp32r) matmuls;
    # split multi-wait instructions into standalone EventSemaphore ops.
    import bass_rust as _bass_rust
    _bass_rust.generate_event_semaphores(nc)
    # Render custom-ISA instructions (e.g. partition_broadcast) to raw bytes.
    mybir.codegen_inst_isa_subclasses(nc)
    return nc


_PROGRAM_CACHE = {}


def get_program():
    if FAST_MM not in _PROGRAM_CACHE:
        _PROGRAM_CACHE[FAST_MM] = build_program()
    return _PROGRAM_CACHE[FAST_MM]


def make_in_maps(x, Wq, bq, Wk, bk, Wv, bv, Wo, bo, gamma, beta):
    x = np.asarray(x, np.float32)
    toT = lambda w: np.ascontiguousarray(np.asarray(w, np.float32).T)
    wqT, wkT, wvT = toT(Wq), toT(Wk), toT(Wv)
    woT = toT(Wo)  # [cin=(h,d), cout]
    woT_ph = np.ascontiguousarray(
        woT.reshape(NH, DH, C).transpose(1, 0, 2).reshape(DH, NH * C))
    b2 = lambda b: np.ascontiguousarray(np.asarray(b, np.float32).reshape(PB, P).T)
    bc = lambda b: np.ascontiguousarray(
        np.broadcast_to(np.asarray(b, np.float32), (P, C)))
    wvT_pad = np.zeros((C, NH * 33), np.float32)
    bv_pad = np.zeros((NH * 33,), np.float32)
    for h in range(NH):
        wvT_pad[:, h * 33: h * 33 + 32] = wvT[:, h * DH: (h + 1) * DH]
        bv_pad[h * 33: h * 33 + 32] = np.asarray(bv, np.float32)[h * DH: (h + 1) * DH]
        bv_pad[h * 33 + 32] = 1.0
    shared = {
        "wqT": wqT, "wkT": wkT, "wvT": wvT, "woT_ph": woT_ph,
        "wvT_pad": wvT_pad,
        "bv_pad_bc": np.ascontiguousarray(np.broadcast_to(bv_pad, (P, NH * 33))),
        "bq2": b2(bq), "bk2": b2(bk),
        "bv_bc": bc(bv), "bo_bc": bc(bo),
        "gamma_bc": bc(gamma), "beta_bc": bc(beta),
    }
    in_maps = []
    for core in range(NCORES):
        b, qh = core // 2, core % 2
        xs = np.asarray(x[b]).reshape(N, C)
        in_maps.append({
            "xsT": np.ascontiguousarray(xs.T),
            "xsTq": np.ascontiguousarray(xs[qh * NQ:(qh + 1) * NQ].T),
            **shared,
        })
    return in_maps


def run(inputs, **kwargs):
    nc = get_program()
    in_maps = make_in_maps(**inputs)
    res = run_bass_kernel_spmd(nc, in_maps, core_ids=list(range(NCORES)), **kwargs)
    full = np.empty((B, N, C), np.float32)
    for core in range(NCORES):
        b, qh = core // 2, core % 2
        full[b, qh * NQ:(qh + 1) * NQ, :] = np.asarray(res.results[core]["out"])
    return full, res


def kernel(**inputs):
    full, _ = run(inputs)
    return full


# revision 18
# speedup vs baseline: 1.5744x; 1.5744x over previous
"""Multi-head self-attention (B=4, N=4096, C=256, nh=8) on 8 trn2 NeuronCores.

Sharding: core c handles batch b=c//2 and query-half qh=c%2 (2048 q tokens,
all 8 heads, full 4096-token K/V context for the batch). Outputs are disjoint
slices of the final [4, 4096, 256] tensor, so gathering is host-side
concatenation. Single SPMD Bass program; per-core behavior comes purely from
per-core input data (no runtime core-id branching).

Per-core pipeline:
  1. Load xs^T (token-transposed host-side), weights (host-pretransposed),
     biases (host-prebroadcast).
  2. Projections on PE: Q^T, K^T as [cout, tokens]; V token-major with an
     interleaved ones-column per head (softmax-denominator trick); vres =
     V projection of the q-half (residual path).
  3. Attention per (head-block pb, q-chunk qc, k-block kb):
       S^T[k,q] = K_h @ Q_h^T on PE, 4 heads packed via tile_position row
                  tiling (contraction K=32 uses 32-row strips of the array)
       P^T = exp(S^T * 1/sqrt(dh)) on ScalarE, PSUM -> SBUF, one op per
             k-block spanning the 4 heads' PSUM banks (free dim 2048)
       O^T[d,q] += [V_h | 1]^T @ P_h^T on PE; row 32 accumulates the softmax
                  denominator for free
     Normalize: DVE reciprocal(denom row) -> GPSIMD partition_broadcast ->
     DVE multiply into per-head attnout tiles.
  4. Output projection (8 per-head K=32 matmuls accumulating in PSUM),
     + residual(vres) + bias, LayerNorm over C on DVE with one batched Sqrt
     on ScalarE (single activation-table switch).
"""

import os
from contextlib import ExitStack

import numpy as np

import concourse.bass as bass
import concourse.tile as tile
from concourse import mybir
from concourse.bass_utils import run_bass_kernel_spmd

F32 = mybir.dt.float32
F32R = mybir.dt.float32r
AF = mybir.ActivationFunctionType
ALU = mybir.AluOpType

B, HH, WW, C = 4, 64, 64, 256
N = HH * WW          # 4096 tokens per batch
NH, DH = 8, 32       # heads, head dim
NCORES = 8
NQ = N // 2          # 2048 q tokens per core
PB = 2               # partition blocks of C (2 x 128)
P = 128
SCALE = 1.0 / float(np.sqrt(DH))
LN_EPS = 1e-6

QC = 512             # q chunk (matmul free dim)
NQC = NQ // QC       # 4
NKB = N // P         # 32 k blocks
NTB = N // P         # 32 token blocks (full batch)
NQB = NQ // P        # 16 token blocks (q half)
VST = NH * 33        # 264: V block layout [128 tok, 8 heads x (32 | ones)]

# Fast matmul mode: stream fp32 operands as float32r (1 cyc/row vs 4).
FAST_MM = os.environ.get("BASS_FAST_MM", "1") == "1"


DT = F32R if FAST_MM else F32   # dtype of matmul-feeding tensors


def build_program():
    nc = bass.Bass()

    d_xsT = nc.declare_dram_parameter("xsT", [C, N], DT, isOutput=False)
    d_xsTq = nc.declare_dram_parameter("xsTq", [C, NQ], DT, isOutput=False)
    d_wqT = nc.declare_dram_parameter("wqT", [C, C], DT, isOutput=False)
    d_wkT = nc.declare_dram_parameter("wkT", [C, C], DT, isOutput=False)
    d_wvT = nc.declare_dram_parameter("wvT", [C, C], DT, isOutput=False)
    d_wvTp = nc.declare_dram_parameter("wvT_pad", [C, VST], DT, isOutput=False)
    d_bvp = nc.declare_dram_parameter("bv_pad_bc", [P, VST], F32, isOutput=False)
    d_woT = nc.declare_dram_parameter("woT_ph", [DH, NH * C], DT, isOutput=False)
    d_bq = nc.declare_dram_parameter("bq2", [P, PB], F32, isOutput=False)
    d_bk = nc.declare_dram_parameter("bk2", [P, PB], F32, isOutput=False)
    d_bv = nc.declare_dram_parameter("bv_bc", [P, C], F32, isOutput=False)
    d_bo = nc.declare_dram_parameter("bo_bc", [P, C], F32, isOutput=False)
    d_gamma = nc.declare_dram_parameter("gamma_bc", [P, C], F32, isOutput=False)
    d_beta = nc.declare_dram_parameter("beta_bc", [P, C], F32, isOutput=False)
    d_out = nc.declare_dram_parameter("out", [NQ, C], F32, isOutput=True)

    with tile.TileContext(nc) as tc, ExitStack() as ctx:
        singles = ctx.enter_context(tc.tile_pool(name="singles", bufs=1))

        wqT_sb = singles.tile([P, PB * C], DT, name="wqT_sb")
        wkT_sb = singles.tile([P, PB * C], DT, name="wkT_sb")
        wvT_sb = singles.tile([P, PB * C], DT, name="wvT_sb")
        wvTp_sb = singles.tile([P, PB * VST], DT, name="wvTp_sb")
        bvp_sb = singles.tile([P, VST], F32, name="bvp_sb")
        woT_sb = singles.tile([DH, NH * C], DT, name="woT_sb")
        bq_sb = singles.tile([P, PB], F32, name="bq_sb")
        bk_sb = singles.tile([P, PB], F32, name="bk_sb")
        bv_sb = singles.tile([P, C], F32, name="bv_sb")
        bo_sb = singles.tile([P, C], F32, name="bo_sb")
        gamma_sb = singles.tile([P, C], F32, name="gamma_sb")
        beta_sb = singles.tile([P, C], F32, name="beta_sb")

        qT_sb = singles.tile([P, PB * NQ], DT, name="qT_sb")
        kT_sb = singles.tile([P, PB * N], DT, name="kT_sb")
        v_sb = singles.tile([P, NTB * VST], DT, name="v_sb")
        vres_sb = singles.tile([P, NQB * C], F32, name="vres_sb")
        stats_sb = singles.tile([P, NQB], F32, name="stats_sb")
        invstd_sb = singles.tile([P, NQB], F32, name="invstd_sb")
        eps_sb = singles.tile([P, 1], F32, name="eps_sb")
        nc.vector.memset(eps_sb[:], LN_EPS)

        for pb in range(PB):
            nc.sync.dma_start(wqT_sb[:, pb * C:(pb + 1) * C], d_wqT[pb * P:(pb + 1) * P, :])
            nc.sync.dma_start(wkT_sb[:, pb * C:(pb + 1) * C], d_wkT[pb * P:(pb + 1) * P, :])
            nc.sync.dma_start(wvT_sb[:, pb * C:(pb + 1) * C], d_wvT[pb * P:(pb + 1) * P, :])
            nc.sync.dma_start(wvTp_sb[:, pb * VST:(pb + 1) * VST], d_wvTp[pb * P:(pb + 1) * P, :])
        nc.sync.dma_start(woT_sb[:], d_woT[:])
        nc.sync.dma_start(bq_sb[:], d_bq[:])
        nc.sync.dma_start(bk_sb[:], d_bk[:])
        nc.sync.dma_start(bv_sb[:], d_bv[:])
        nc.sync.dma_start(bvp_sb[:], d_bvp[:])
        nc.sync.dma_start(bo_sb[:], d_bo[:])
        nc.sync.dma_start(gamma_sb[:], d_gamma[:])
        nc.sync.dma_start(beta_sb[:], d_beta[:])

        # ones-columns come from the padded V projection: wvT_pad has a zero
        # column at each head's 33rd slot and bv_pad_bc carries 1.0 there.

        # ---- phase 1: projections (xs pool released afterwards) ----
        with tc.tile_pool(name="xs_pool", bufs=1) as xs_pool, \
             tc.tile_pool(name="proj_psum", bufs=2, space="PSUM") as proj_psum:
            xsT_sb = xs_pool.tile([P, PB * N], DT, name="xsT_sb")
            xsTq_sb = xs_pool.tile([P, PB * NQ], DT, name="xsTq_sb")
            for pb in range(PB):
                nc.sync.dma_start(xsT_sb[:, pb * N:(pb + 1) * N],
                                  d_xsT[pb * P:(pb + 1) * P, :])
                nc.sync.dma_start(xsTq_sb[:, pb * NQ:(pb + 1) * NQ],
                                  d_xsTq[pb * P:(pb + 1) * P, :])

            # K^T and Q^T: [cout(part), tokens]
            for (wsb, bsb, dst, ntok, src) in (
                (wkT_sb, bk_sb, kT_sb, N, xsT_sb),
                (wqT_sb, bq_sb, qT_sb, NQ, xsTq_sb),
            ):
                for pbo in range(PB):
                    for t0 in range(0, ntok, QC):
                        pt = proj_psum.tile([P, QC], F32, name="pt", tag="pt")
                        for pbi in range(PB):
                            nc.tensor.matmul(
                                pt[:],
                                (wsb[:, pbi * C + pbo * P: pbi * C + (pbo + 1) * P]),
                                (src[:, pbi * ntok + t0: pbi * ntok + t0 + QC]),
                                start=(pbi == 0), stop=(pbi == PB - 1),
                            )
                        nc.vector.tensor_scalar_add(
                            dst[:, pbo * ntok + t0: pbo * ntok + t0 + QC],
                            pt[:], bsb[:, pbo:pbo + 1],
                        )

            # V token-major over the full batch (33-stride head layout)
            for tb in range(NTB):
                vt = proj_psum.tile([P, VST], F32, name="vt", tag="vt")
                for pbi in range(PB):
                    nc.tensor.matmul(
                        vt[:],
                        (xsT_sb[:, pbi * N + tb * P: pbi * N + (tb + 1) * P]),
                        (wvTp_sb[:, pbi * VST:(pbi + 1) * VST]),
                        start=(pbi == 0), stop=(pbi == PB - 1),
                    )
                nc.vector.scalar_tensor_tensor(
                    v_sb[:, tb * VST:(tb + 1) * VST],
                    vt[:], 0.0, bvp_sb[:], ALU.bypass, ALU.add,
                )
            # vres: V of the q-half tokens, plain token-major layout
            for tb in range(NQB):
                vr = proj_psum.tile([P, C], F32, name="vr", tag="vt")
                for pbi in range(PB):
                    nc.tensor.matmul(
                        vr[:],
                        (xsTq_sb[:, pbi * NQ + tb * P: pbi * NQ + (tb + 1) * P]),
                        (wvT_sb[:, pbi * C:(pbi + 1) * C]),
                        start=(pbi == 0), stop=(pbi == PB - 1),
                    )
                nc.vector.tensor_add(vres_sb[:, tb * C:(tb + 1) * C], vr[:], bv_sb[:])

        # ---- phase 2: attention + out-proj + residual + LN stats, per q-chunk ----
        # Score PSUM is a ring of 2-bank tiles (bufs=2 -> 4 banks) so PE can
        # write scores for slot s+1 while ScalarE exps slot s; 4 PV banks.
        with tc.tile_pool(name="score_psum", bufs=2, space="PSUM") as score_psum, \
             tc.tile_pool(name="pv_psum", bufs=1, space="PSUM") as pv_psum, \
             tc.tile_pool(name="p_pool", bufs=4) as p_pool, \
             tc.tile_pool(name="attn_pool", bufs=1) as attn_pool, \
             tc.tile_pool(name="norm_pool", bufs=2) as norm_pool, \
             tc.tile_pool(name="dram_pool", bufs=4, space="DRAM") as dram_pool, \
             tc.tile_pool(name="out_pool", bufs=3) as out_pool, \
             tc.tile_pool(name="xc_pool", bufs=NQB) as xc_pool:
            RSLOTS = 2  # score sub-slots per ring tile (ACT op free dim 1024)
            ring = {"st": None, "pt": None, "n": 0, "items": []}

            def ring_flush():
                if ring["n"] == 0:
                    return
                w = ring["n"] * QC
                nc.scalar.activation(ring["pt"][:, :w], ring["st"][:, :w],
                                     AF.Exp, scale=SCALE)
                for idx, (h, kb, pvt) in enumerate(ring["items"]):
                    nc.tensor.matmul(
                        pvt[:],
                        (v_sb[:, kb * VST + h * 33: kb * VST + h * 33 + 33]),
                        (ring["pt"][:, idx * QC:(idx + 1) * QC]),
                        start=(kb == 0), stop=(kb == NKB - 1),
                    )
                ring.update(st=None, pt=None, n=0, items=[])

            xcs = []
            for qc in range(NQC):
                attn_t = attn_pool.tile([DH, NH * QC], DT, name="attn_t", tag="attn")
                for pb in range(PB):
                    q0 = pb * NQ + qc * QC
                    pv = [pv_psum.tile([DH + 1, QC], F32, name=f"pv{j}", tag=f"pv{j}")
                          for j in range(4)]
                    for kb in range(NKB):
                        for j in range(4):
                            h = pb * 4 + j
                            if ring["st"] is None:
                                ring["st"] = score_psum.tile(
                                    [P, RSLOTS * QC], F32, name="st", tag="st")
                                ring["pt"] = p_pool.tile(
                                    [P, RSLOTS * QC], DT, name="ptile", tag="ptile")
                            idx = ring["n"]
                            nc.tensor.matmul(
                                ring["st"][:, idx * QC:(idx + 1) * QC],
                                (kT_sb[j * DH:(j + 1) * DH,
                                       pb * N + kb * P: pb * N + (kb + 1) * P]),
                                (qT_sb[j * DH:(j + 1) * DH, q0: q0 + QC]),
                                start=True, stop=True,
                                tile_position=(j * DH, 0),
                            )
                            ring["items"].append((h, kb, pv[j]))
                            ring["n"] += 1
                            if ring["n"] == RSLOTS:
                                ring_flush()
                    ring_flush()  # (pb, qc) boundary: pv normalize needs all PV
                    for j in range(4):
                        h = pb * 4 + j
                        recip = norm_pool.tile([DH + 1, QC], F32, name="recip", tag="recip")
                        nc.vector.reciprocal(recip[32:33, :], pv[j][32:33, :])
                        # partition-broadcast via DRAM bounce (SBUF APs cannot
                        # have zero partition step; DRAM APs can)
                        dscr = dram_pool.tile([1, QC], F32, name="dscr", tag="dscr")
                        nc.sync.dma_start(dscr[:], recip[32:33, :])
                        bcast = norm_pool.tile([DH, QC], F32, name="bcast", tag="bcast")
                        nc.sync.dma_start(bcast[:], dscr[:].to_broadcast([DH, QC]))
                        nc.vector.tensor_mul(
                            attn_t[:, h * QC:(h + 1) * QC], pv[j][0:32, :], bcast[:],
                        )
                # out-proj + residual + LN stats for this q-chunk's 4 token blocks
                for tl in range(QC // P):
                    tb = qc * (QC // P) + tl
                    # reuse a PV psum slot (dead after normalize) for out-proj
                    ot = pv_psum.tile([P, C], F32, name="ot", tag=f"pv{tl}")
                    for h in range(NH):
                        nc.tensor.matmul(
                            ot[:],
                            (attn_t[:, h * QC + tl * P: h * QC + (tl + 1) * P]),
                            (woT_sb[:, h * C:(h + 1) * C]),
                            start=(h == 0), stop=(h == NH - 1),
                        )
                    o1 = out_pool.tile([P, C], F32, name="o1", tag="o1")
                    nc.vector.tensor_add(o1[:], ot[:], vres_sb[:, tb * C:(tb + 1) * C])
                    nc.vector.tensor_add(o1[:], o1[:], bo_sb[:])
                    sum_t = out_pool.tile([P, 1], F32, name="sum_t", tag="sum_t")
                    nc.vector.tensor_reduce(sum_t[:], o1[:], mybir.AxisListType.X, ALU.add)
                    nm = out_pool.tile([P, 1], F32, name="nm", tag="nm")
                    nc.vector.tensor_scalar_mul(nm[:], sum_t[:], -1.0 / C)
                    xc = xc_pool.tile([P, C], F32, name="xc", tag="xc")
                    nc.vector.tensor_scalar_add(xc[:], o1[:], nm[:])
                    sq = out_pool.tile([P, C], F32, name="sq", tag="sq")
                    nc.vector.scalar_tensor_tensor(
                        sq[:], xc[:], 0.0, xc[:], ALU.bypass, ALU.mult,
                        accum_out=stats_sb[:, tb:tb + 1],
                    )
                    xcs.append(xc)
            # ---- LN finalize (one batched Sqrt -> one activation-table switch) ----
            std_t = singles.tile([P, NQB], F32, name="std_t")
            nc.scalar.activation(std_t[:], stats_sb[:], AF.Sqrt,
                                 bias=eps_sb[:], scale=1.0 / C)
            nc.vector.reciprocal(invstd_sb[:], std_t[:])
            for tb in range(NQB):
                f1 = out_pool.tile([P, C], F32, name="f1", tag="f1")
                nc.vector.scalar_tensor_tensor(
                    f1[:], xcs[tb][:], invstd_sb[:, tb:tb + 1], gamma_sb[:],
                    ALU.mult, ALU.mult,
                )
                f2 = out_pool.tile([P, C], F32, name="f2", tag="f2")
                nc.vector.tensor_add(f2[:], f1[:], beta_sb[:])
                nc.sync.dma_start(d_out[tb * P:(tb + 1) * P, :], f2[:])

    # Walrus allows only 1 sync wait on self-loading (fp32/fp32r) matmuls;
    # split multi-wait instructions into standalone EventSemaphore ops.
    import bass_rust as _bass_rust
    _bass_rust.generate_event_semaphores(nc)
    # Render custom-ISA instructions (e.g. partition_broadcast) to raw bytes.
    mybir.codegen_inst_isa_subclasses(nc)
    return nc


_PROGRAM_CACHE = {}


def get_program():
    if FAST_MM not in _PROGRAM_CACHE:
        _PROGRAM_CACHE[FAST_MM] = build_program()
    return _PROGRAM_CACHE[FAST_MM]


def make_in_maps(x, Wq, bq, Wk, bk, Wv, bv, Wo, bo, gamma, beta):
    x = np.asarray(x, np.float32)
    toT = lambda w: np.ascontiguousarray(np.asarray(w, np.float32).T)
    wqT, wkT, wvT = toT(Wq), toT(Wk), toT(Wv)
    woT = toT(Wo)  # [cin=(h,d), cout]
    woT_ph = np.ascontiguousarray(
        woT.reshape(NH, DH, C).transpose(1, 0, 2).reshape(DH, NH * C))
    b2 = lambda b: np.ascontiguousarray(np.asarray(b, np.float32).reshape(PB, P).T)
    bc = lambda b: np.ascontiguousarray(
        np.broadcast_to(np.asarray(b, np.float32), (P, C)))
    wvT_pad = np.zeros((C, NH * 33), np.float32)
    bv_pad = np.zeros((NH * 33,), np.float32)
    for h in range(NH):
        wvT_pad[:, h * 33: h * 33 + 32] = wvT[:, h * DH: (h + 1) * DH]
        bv_pad[h * 33: h * 33 + 32] = np.asarray(bv, np.float32)[h * DH: (h + 1) * DH]
        bv_pad[h * 33 + 32] = 1.0
    shared = {
        "wqT": wqT, "wkT": wkT, "wvT": wvT, "woT_ph": woT_ph,
        "wvT_pad": wvT_pad,
        "bv_pad_bc": np.ascontiguousarray(np.broadcast_to(bv_pad, (P, NH * 33))),
        "bq2": b2(bq), "bk2": b2(bk),
        "bv_bc": bc(bv), "bo_bc": bc(bo),
        "gamma_bc": bc(gamma), "beta_bc": bc(beta),
    }
    in_maps = []
    for core in range(NCORES):
        b, qh = core // 2, core % 2
        xs = np.asarray(x[b]).reshape(N, C)
        in_maps.append({
            "xsT": np.ascontiguousarray(xs.T),
            "xsTq": np.ascontiguousarray(xs[qh * NQ:(qh + 1) * NQ].T),
            **shared,
        })
    return in_maps


def run(inputs, **kwargs):
    nc = get_program()
    in_maps = make_in_maps(**inputs)
    res = run_bass_kernel_spmd(nc, in_maps, core_ids=list(range(NCORES)), **kwargs)
    full = np.empty((B, N, C), np.float32)
    for core in range(NCORES):
        b, qh = core // 2, core % 2
        full[b, qh * NQ:(qh + 1) * NQ, :] = np.asarray(res.results[core]["out"])
    return full, res


def kernel(**inputs):
    full, _ = run(inputs)
    return full
